# revision 15
# baseline (speedup 1.0000x reference)
"""DistanceAttention Trainium2 kernel.

Full inputs -> full outputs (output, attn). Batch (B=8) is sharded across the
8 NeuronCores, one batch element per core. Weights are replicated.

Per-core dataflow:
  - PE-transpose X_{q,k,v} -> X^T  (fp32)
  - projections (float32r matmuls): Q^T [Hd, S], K^T [Hd, S], V natural (bf16)
  - per (head, q-tile): S = Q_h^T.T @ K_h^T  (PSUM, float32r, N=512)
        u  = (S * 1/8) * dist          (DVE scalar_tensor_tensor, PSUM src)
        e  = exp(u), rowsum            (ACT activation Exp + accum_out)
        r  = 1/rowsum                  (DVE reciprocal)
        P  = e * r                     (DVE tensor_scalar, 2x mode) -> DMA attn
        P^T: 8 PE fp32 transposes -> one drain (cast to bf16, 3D AP)
        PV: 8 bf16 matmuls accumulate C^T[:, qt window] per head
  - per head: drain C^T -> bf16
  - output projection (bf16) + fp32 residual add -> out
"""
import numpy as np

import concourse.bass as bass
import concourse.bacc as bacc
import concourse.tile as tile
from concourse import mybir
from concourse.bass_utils import run_bass_kernel_spmd
from concourse.masks import make_identity

B, S, D = 8, 1024, 512
H, DH = 8, 64
N_CORES = 8
SCALE = 0.125  # 1/sqrt(64)

f32 = mybir.dt.float32
f32r = mybir.dt.float32r
bf16 = mybir.dt.bfloat16

_CACHE = {}


def build(use_mask: bool, mm_dt=f32r):
    nc = bacc.Bacc()

    xq_d = nc.dram_tensor("xq", [S, D], f32, kind="ExternalInput")
    xk_d = nc.dram_tensor("xk", [S, D], f32, kind="ExternalInput")
    xv_d = nc.dram_tensor("xv", [S, D], f32, kind="ExternalInput")
    dist_d = nc.dram_tensor("dist", [S, S], f32, kind="ExternalInput")
    wq_d = nc.dram_tensor("wq", [D, D], f32, kind="ExternalInput")
    wk_d = nc.dram_tensor("wk", [D, D], f32, kind="ExternalInput")
    wv_d = nc.dram_tensor("wv", [D, D], f32, kind="ExternalInput")
    wo_d = nc.dram_tensor("wo", [D, D], f32, kind="ExternalInput")
    if use_mask:
        madd_d = nc.dram_tensor("madd", [S, S], f32, kind="ExternalInput")
    out_d = nc.dram_tensor("out", [S, D], f32, kind="ExternalOutput")
    attn_d = nc.dram_tensor("attn", [H, S, S], f32, kind="ExternalOutput")

    NQ = S // 128      # 8 q-tiles
    NC = D // 128      # 4 contraction chunks of D
    NK = S // 128      # 8 k-chunks

    def drain(i, out, in_):
        """PSUM -> SBUF copy (casts to out dtype), alternating ACT/DVE."""
        if i % 2 == 0:
            nc.scalar.copy(out, in_)
        else:
            nc.vector.tensor_copy(out, in_)

    with tile.TileContext(nc) as tc:
        with (
            tc.tile_pool(name="const", bufs=1) as constp,
            tc.tile_pool(name="resident", bufs=1) as resp,
            tc.tile_pool(name="stage", bufs=1) as stagep,
            tc.tile_pool(name="stage2", bufs=2) as stagep2,
            tc.tile_pool(name="work", bufs=2) as workp,
            tc.tile_pool(name="work3", bufs=3) as workp3,
            tc.tile_pool(name="work4", bufs=6) as workp4,
            tc.tile_pool(name="ps_big", bufs=3, space="PSUM") as ps_big,
            tc.tile_pool(name="ps_c", bufs=1, space="PSUM") as ps_c,
            tc.tile_pool(name="dram", bufs=2, space="DRAM") as drampool,
        ):
            # ---- constants ----
            id32 = constp.tile([128, 128], f32)
            make_identity(nc, id32)

            # ---- resident tensors ----
            dist_sb = resp.tile([128, NQ, S], f32, tag="dist")   # 32KB/part
            nc.sync.dma_start(
                dist_sb[:], dist_d.rearrange("(c p) k -> p c k", p=128))
            wo_bf = resp.tile([128, NC, D], bf16, tag="wo")      # 4KB/part
            nc.gpsimd.dma_start(
                out=wo_bf[:], in_=wo_d.rearrange("(c p) d -> p c d", p=128))

            qt_sb = resp.tile([128, NC, S], mm_dt, tag="qt")     # 16KB/part
            kt_sb = resp.tile([128, NC, S], mm_dt, tag="kt")     # 16KB/part
            v_bf = resp.tile([128, NK, D], bf16, tag="v")        # 8KB/part

            # ---- stage A+B: transpose X, project ----
            dcnt = 0
            for name, x_d, w_d, dst in (
                ("q", xq_d, wq_d, qt_sb),
                ("k", xk_d, wk_d, kt_sb),
                ("v", xv_d, wv_d, v_bf),
            ):
                w_sb = stagep.tile([128, NC, D], mm_dt, tag="w_stage")
                nc.gpsimd.dma_start(
                    out=w_sb[:], in_=w_d.rearrange("(c p) d -> p c d", p=128))
                xt_sb = stagep.tile([128, NC, S], mm_dt, tag="xt_stage")
                # transpose x [S, D] -> x^T stored as [128, NC, S]
                for si in range(NQ):
                    xst = stagep2.tile([128, D], f32, tag="x_stage")
                    nc.sync.dma_start(
                        xst[:], x_d[si * 128:(si + 1) * 128, :])
                    x_tile = xst[:]
                    tp = ps_big.tile([128, S], f32, tag="big")
                    for ci in range(NC):
                        nc.tensor.transpose(
                            tp[:, ci * 128:(ci + 1) * 128],
                            x_tile[:, ci * 128:(ci + 1) * 128],
                            id32[:],
                        )
                    # one drain for the 4 blocks (3D out AP)
                    drain(dcnt, xt_sb[:, :, si * 128:(si + 1) * 128],
                          tp[:, 0:512].rearrange("p (c q) -> p c q", c=NC))
                    dcnt += 1
                # projection
                if name in ("q", "k"):
                    # dst[:, mc, s] = sum_d w[d, mc-slice] * x^T[d, s]
                    for mc in range(NC):
                        for nh in range(2):
                            pp = ps_big.tile([128, S], f32, tag="big")
                            for kc in range(NC):
                                nc.tensor.matmul(
                                    pp[:, 0:512],
                                    w_sb[:, kc, mc * 128:(mc + 1) * 128],
                                    xt_sb[:, kc, nh * 512:(nh + 1) * 512],
                                    start=(kc == 0), stop=(kc == NC - 1),
                                )
                            drain(dcnt, dst[:, mc, nh * 512:(nh + 1) * 512],
                                  pp[:, 0:512])
                            dcnt += 1
                else:
                    # V natural: dst[:, sc, hdv] = sum_d x^T[d, sc-slice] * w[d, :]
                    for sc in range(NK):
                        pp = ps_big.tile([128, S], f32, tag="big")
                        for kc in range(NC):
                            nc.tensor.matmul(
                                pp[:, 0:512],
                                xt_sb[:, kc, sc * 128:(sc + 1) * 128],
                                w_sb[:, kc, :],
                                start=(kc == 0), stop=(kc == NC - 1),
                            )
                        drain(dcnt, dst[:, sc, :], pp[:, 0:512])
                        dcnt += 1

            # ---- stage C: attention ----
            ct_sb = resp.tile([128, NC, S], bf16, tag="ct")      # 8KB/part

            def pv_for_head(ph, pt_tile):
                cp = ps_c.tile([64, S], f32, tag="ps_cs")
                for qh in range(2):
                    for kc in range(NK):
                        nc.tensor.matmul(
                            cp[:, qh * 512:(qh + 1) * 512],
                            v_bf[:, kc, ph * 64:(ph + 1) * 64],
                            pt_tile[:, kc, qh * 512:(qh + 1) * 512],
                            start=(kc == 0), stop=(kc == NK - 1),
                        )
                return cp

            def ct_drain(ph, cp, qh):
                pmc, ppo = ph // 2, (ph % 2) * 64
                drain(ph + qh,
                      ct_sb[ppo:ppo + 64, pmc, qh * 512:(qh + 1) * 512],
                      cp[:, qh * 512:(qh + 1) * 512])

            prev = None  # (head, pb_dram, pt_tile) of previous head
            pvq = None   # (head, cp) whose C^T drains are still pending
            pend = None  # deferred epilogue of previous (h, qt) iteration
            for h in range(H):
                mc, po = h // 2, (h % 2) * 64
                pb_dram = drampool.tile([S, S], bf16, tag="pb")
                pt_cur = workp.tile([128, NK, S], bf16, tag="pt")
                for qt in range(NQ):
                    s_ps = ps_big.tile([128, S], f32, tag="big")
                    for kh in range(2):
                        nc.tensor.matmul(
                            s_ps[:, kh * 512:(kh + 1) * 512],
                            qt_sb[po:po + 64, mc, qt * 128:(qt + 1) * 128],
                            kt_sb[po:po + 64, mc, kh * 512:(kh + 1) * 512],
                            start=True, stop=True,
                        )
                    u_sb = workp3.tile([128, S], f32, tag="u")
                    nc.vector.scalar_tensor_tensor(
                        u_sb[:], s_ps[:], SCALE, dist_sb[:, qt, :],
                        op0=mybir.AluOpType.mult, op1=mybir.AluOpType.mult)
                    if use_mask:
                        m_sb = workp.tile([128, S], f32, tag="madd")
                        nc.sync.dma_start(
                            m_sb[:], madd_d[qt * 128:(qt + 1) * 128, :])
                        nc.vector.tensor_add(u_sb[:], u_sb[:], m_sb[:])
                    e_sb = workp3.tile([128, S], f32, tag="e")
                    rowsum = workp3.tile([128, 1], f32, tag="rowsum")
                    nc.scalar.activation(
                        e_sb[:], u_sb[:], mybir.ActivationFunctionType.Exp,
                        accum_out=rowsum[:])
                    # epilogue of the PREVIOUS iteration (skewed by one to
                    # keep DVE from waiting on this iteration's exp)
                    if pend is not None:
                        _h, _qt, _e, _rs, _pb = pend
                        recip = workp3.tile([128, 1], f32, tag="recip")
                        nc.vector.reciprocal(recip[:], _rs[:])
                        p_sb = workp4.tile([128, S], f32, tag="p")
                        nc.vector.tensor_scalar(
                            p_sb[:], _e[:], recip[:], None,
                            op0=mybir.AluOpType.mult)
                        nc.gpsimd.dma_start(
                            out=attn_d[_h, _qt * 128:(_qt + 1) * 128, :],
                            in_=p_sb[:])
                        nc.gpsimd.dma_start(
                            out=_pb[_qt * 128:(_qt + 1) * 128, :], in_=p_sb[:])
                    pend = (h, qt, e_sb, rowsum, pb_dram)
                    # previous-previous head's C^T drains (PV long done)
                    if pvq and qt < 2:
                        ct_drain(pvq[0], pvq[1], qt)
                    # interleave previous head's transpose-reads
                    if prev is not None:
                        nc.sync.dma_start_transpose(
                            prev[2][:, qt, :],
                            prev[1][:, qt * 128:(qt + 1) * 128])
                if prev is not None:
                    cp = pv_for_head(prev[0], prev[2])
                    pvq = (prev[0], cp)
                prev = (h, pb_dram, pt_cur)
            # epilogue: flush last pending iteration
            _h, _qt, _e, _rs, _pb = pend
            recip = workp3.tile([128, 1], f32, tag="recip")
            nc.vector.reciprocal(recip[:], _rs[:])
            p_sb = workp4.tile([128, S], f32, tag="p")
            nc.vector.tensor_scalar(
                p_sb[:], _e[:], recip[:], None, op0=mybir.AluOpType.mult)
            nc.sync.dma_start(
                attn_d[_h, _qt * 128:(_qt + 1) * 128, :], p_sb[:])
            nc.gpsimd.dma_start(
                out=_pb[_qt * 128:(_qt + 1) * 128, :], in_=p_sb[:])
            # last head transposes + PV + remaining drains
            if pvq:
                ct_drain(pvq[0], pvq[1], 0)
                ct_drain(pvq[0], pvq[1], 1)
            for kc in range(NK):
                nc.sync.dma_start_transpose(
                    prev[2][:, kc, :], prev[1][:, kc * 128:(kc + 1) * 128])
            cp = pv_for_head(prev[0], prev[2])
            ct_drain(prev[0], cp, 0)
            ct_drain(prev[0], cp, 1)

            # ---- stage D: output projection + residual ----
            for st in range(NQ):
                op = ps_big.tile([128, S], f32, tag="big")
                for kc in range(NC):
                    nc.tensor.matmul(
                        op[:, 0:512],
                        ct_sb[:, kc, st * 128:(st + 1) * 128],
                        wo_bf[:, kc, :],
                        start=(kc == 0), stop=(kc == NC - 1),
                    )
                xq_res = workp.tile([128, D], f32, tag="xq_res")
                nc.sync.dma_start(
                    xq_res[:], xq_d[st * 128:(st + 1) * 128, :])
                o_sb = workp.tile([128, D], f32, tag="o")
                nc.vector.tensor_add(o_sb[:], op[:, 0:512], xq_res[:])
                nc.sync.dma_start(out_d[st * 128:(st + 1) * 128, :], o_sb[:])

    nc.compile()
    return nc


def _get_nc(use_mask):
    key = ("nc", use_mask)
    if key not in _CACHE:
        _CACHE[key] = build(use_mask)
    return _CACHE[key]


def kernel(input_Q, input_K, input_V, dist_factor, attn_mask,
           W_Q, W_K, W_V, W_O):
    input_Q = np.ascontiguousarray(np.asarray(input_Q, dtype=np.float32))
    input_K = np.ascontiguousarray(np.asarray(input_K, dtype=np.float32))
    input_V = np.ascontiguousarray(np.asarray(input_V, dtype=np.float32))
    dist_factor = np.ascontiguousarray(np.asarray(dist_factor, dtype=np.float32))
    attn_mask = np.asarray(attn_mask)
    W_Q = np.ascontiguousarray(np.asarray(W_Q, dtype=np.float32))
    W_K = np.ascontiguousarray(np.asarray(W_K, dtype=np.float32))
    W_V = np.ascontiguousarray(np.asarray(W_V, dtype=np.float32))
    W_O = np.ascontiguousarray(np.asarray(W_O, dtype=np.float32))

    use_mask = bool(attn_mask.any())
    nc = _get_nc(use_mask)

    in_maps = []
    for c in range(N_CORES):
        m = {
            "xq": input_Q[c], "xk": input_K[c], "xv": input_V[c],
            "dist": dist_factor[c],
            "wq": W_Q, "wk": W_K, "wv": W_V, "wo": W_O,
        }
        if use_mask:
            m["madd"] = np.where(attn_mask[c], np.float32(-1e10),
                                 np.float32(0.0)).astype(np.float32)
        in_maps.append(m)

    res = run_bass_kernel_spmd(nc, in_maps, core_ids=list(range(N_CORES)))
    output = np.stack([res.results[c]["out"] for c in range(N_CORES)])
    attn = np.stack([res.results[c]["attn"] for c in range(N_CORES)])
    return output, attn


# revision 16
# speedup vs baseline: 2.5163x; 2.5163x over previous
"""DistanceAttention Trainium2 kernel.

Full inputs -> full outputs (output, attn). Batch (B=8) is sharded across the
8 NeuronCores, one batch element per core. Weights are replicated.

Per-core dataflow:
  - PE-transpose X_{q,k,v} -> X^T  (fp32)
  - projections (float32r matmuls): Q^T [Hd, S], K^T [Hd, S], V natural (bf16)
  - per (head, q-tile): S = Q_h^T.T @ K_h^T  (PSUM, float32r, N=512)
        u  = (S * 1/8) * dist          (DVE scalar_tensor_tensor, PSUM src)
        e  = exp(u), rowsum            (ACT activation Exp + accum_out)
        r  = 1/rowsum                  (DVE reciprocal)
        P  = e * r                     (DVE tensor_scalar, 2x mode) -> DMA attn
        P^T: 8 PE fp32 transposes -> one drain (cast to bf16, 3D AP)
        PV: 8 bf16 matmuls accumulate C^T[:, qt window] per head
  - per head: drain C^T -> bf16
  - output projection (bf16) + fp32 residual add -> out
"""
import numpy as np

import concourse.bass as bass
import concourse.bacc as bacc
import concourse.tile as tile
from concourse import mybir
from concourse.bass_utils import run_bass_kernel_spmd
from concourse.masks import make_identity

B, S, D = 8, 1024, 512
H, DH = 8, 64
N_CORES = 8
SCALE = 0.125  # 1/sqrt(64)

f32 = mybir.dt.float32
f32r = mybir.dt.float32r
bf16 = mybir.dt.bfloat16

_CACHE = {}


def build(use_mask: bool, mm_dt=f32r, probe=0):
    nc = bacc.Bacc()

    xq_d = nc.dram_tensor("xq", [S, D], f32, kind="ExternalInput")
    xk_d = nc.dram_tensor("xk", [S, D], f32, kind="ExternalInput")
    xv_d = nc.dram_tensor("xv", [S, D], f32, kind="ExternalInput")
    dist_d = nc.dram_tensor("dist", [S, S], f32, kind="ExternalInput")
    wq_d = nc.dram_tensor("wq", [D, D], f32, kind="ExternalInput")
    wk_d = nc.dram_tensor("wk", [D, D], f32, kind="ExternalInput")
    wv_d = nc.dram_tensor("wv", [D, D], f32, kind="ExternalInput")
    wo_d = nc.dram_tensor("wo", [D, D], f32, kind="ExternalInput")
    if use_mask:
        madd_d = nc.dram_tensor("madd", [S, S], f32, kind="ExternalInput")
    out_d = nc.dram_tensor("out", [S, D], f32, kind="ExternalOutput")
    attn_d = nc.dram_tensor("attn", [H, S, S], f32, kind="ExternalOutput")

    NQ = S // 128      # 8 q-tiles
    NC = D // 128      # 4 contraction chunks of D
    NK = S // 128      # 8 k-chunks

    def drain(i, out, in_):
        """PSUM -> SBUF copy (casts to out dtype), alternating ACT/DVE."""
        if i % 2 == 0:
            nc.scalar.copy(out, in_)
        else:
            nc.vector.tensor_copy(out, in_)

    with tile.TileContext(nc) as tc:
        with (
            tc.tile_pool(name="const", bufs=1) as constp,
            tc.tile_pool(name="resident", bufs=1) as resp,
            tc.tile_pool(name="stage", bufs=1) as stagep,
            tc.tile_pool(name="stage2", bufs=2) as stagep2,
            tc.tile_pool(name="work", bufs=2) as workp,
            tc.tile_pool(name="work3", bufs=3) as workp3,
            tc.tile_pool(name="work4", bufs=6) as workp4,
            tc.tile_pool(name="ps_big", bufs=3, space="PSUM") as ps_big,
            tc.tile_pool(name="ps_c", bufs=1, space="PSUM") as ps_c,
            tc.tile_pool(name="dram", bufs=2, space="DRAM") as drampool,
        ):
            # ---- constants ----
            id32 = constp.tile([128, 128], f32)
            make_identity(nc, id32)

            # ---- resident tensors ----
            dist_sb = resp.tile([128, NQ, S], f32, tag="dist")   # 32KB/part
            nc.sync.dma_start(
                dist_sb[:], dist_d.rearrange("(c p) k -> p c k", p=128))
            wo_bf = resp.tile([128, NC, D], bf16, tag="wo")      # 4KB/part
            nc.gpsimd.dma_start(
                out=wo_bf[:], in_=wo_d.rearrange("(c p) d -> p c d", p=128))

            qt_sb = resp.tile([128, NC, S], mm_dt, tag="qt")     # 16KB/part
            kt_sb = resp.tile([128, NC, S], mm_dt, tag="kt")     # 16KB/part
            v_bf = resp.tile([128, NK, D], bf16, tag="v")        # 8KB/part

            # ---- stage A+B: transpose X, project ----
            dcnt = 0
            for name, x_d, w_d, dst in (
                ("q", xq_d, wq_d, qt_sb),
                ("k", xk_d, wk_d, kt_sb),
                ("v", xv_d, wv_d, v_bf),
            ):
                w_sb = stagep.tile([128, NC, D], mm_dt, tag="w_stage")
                nc.gpsimd.dma_start(
                    out=w_sb[:], in_=w_d.rearrange("(c p) d -> p c d", p=128))
                xt_sb = stagep.tile([128, NC, S], mm_dt, tag="xt_stage")
                # transpose x [S, D] -> x^T stored as [128, NC, S]
                for si in range(NQ):
                    xst = stagep2.tile([128, D], f32, tag="x_stage")
                    nc.sync.dma_start(
                        xst[:], x_d[si * 128:(si + 1) * 128, :])
                    x_tile = xst[:]
                    tp = ps_big.tile([128, S], f32, tag="big")
                    for ci in range(NC):
                        nc.tensor.transpose(
                            tp[:, ci * 128:(ci + 1) * 128],
                            x_tile[:, ci * 128:(ci + 1) * 128],
                            id32[:],
                        )
                    # one drain for the 4 blocks (3D out AP)
                    drain(dcnt, xt_sb[:, :, si * 128:(si + 1) * 128],
                          tp[:, 0:512].rearrange("p (c q) -> p c q", c=NC))
                    dcnt += 1
                # projection
                if name in ("q", "k"):
                    # dst[:, mc, s] = sum_d w[d, mc-slice] * x^T[d, s]
                    for mc in range(NC):
                        for nh in range(2):
                            pp = ps_big.tile([128, S], f32, tag="big")
                            for kc in range(NC):
                                nc.tensor.matmul(
                                    pp[:, 0:512],
                                    w_sb[:, kc, mc * 128:(mc + 1) * 128],
                                    xt_sb[:, kc, nh * 512:(nh + 1) * 512],
                                    start=(kc == 0), stop=(kc == NC - 1),
                                )
                            drain(dcnt, dst[:, mc, nh * 512:(nh + 1) * 512],
                                  pp[:, 0:512])
                            dcnt += 1
                else:
                    # V natural: dst[:, sc, hdv] = sum_d x^T[d, sc-slice] * w[d, :]
                    for sc in range(NK):
                        pp = ps_big.tile([128, S], f32, tag="big")
                        for kc in range(NC):
                            nc.tensor.matmul(
                                pp[:, 0:512],
                                xt_sb[:, kc, sc * 128:(sc + 1) * 128],
                                w_sb[:, kc, :],
                                start=(kc == 0), stop=(kc == NC - 1),
                            )
                        drain(dcnt, dst[:, sc, :], pp[:, 0:512])
                        dcnt += 1

            # ---- stage C: attention ----
            ct_sb = resp.tile([128, NC, S], bf16, tag="ct")      # 8KB/part

            def pv_for_head(ph, pt_tile):
                cp = ps_c.tile([64, S], f32, tag="ps_cs")
                for qh in range(2):
                    for kc in range(NK):
                        nc.tensor.matmul(
                            cp[:, qh * 512:(qh + 1) * 512],
                            v_bf[:, kc, ph * 64:(ph + 1) * 64],
                            pt_tile[:, kc, qh * 512:(qh + 1) * 512],
                            start=(kc == 0), stop=(kc == NK - 1),
                        )
                return cp

            def ct_drain(ph, cp, qh):
                pmc, ppo = ph // 2, (ph % 2) * 64
                drain(ph + qh,
                      ct_sb[ppo:ppo + 64, pmc, qh * 512:(qh + 1) * 512],
                      cp[:, qh * 512:(qh + 1) * 512])

            prev = None  # (head, pb_dram, pt_tile) of previous head
            pvq = None   # (head, cp) whose C^T drains are still pending
            pend = None  # deferred epilogue of previous (h, qt) iteration
            for h in range(H):
                mc, po = h // 2, (h % 2) * 64
                pb_dram = drampool.tile([S, S], bf16, tag="pb")
                pt_cur = workp.tile([128, NK, S], bf16, tag="pt")
                for qt in range(NQ):
                    s_ps = ps_big.tile([128, S], f32, tag="big")
                    for kh in range(2):
                        nc.tensor.matmul(
                            s_ps[:, kh * 512:(kh + 1) * 512],
                            qt_sb[po:po + 64, mc, qt * 128:(qt + 1) * 128],
                            kt_sb[po:po + 64, mc, kh * 512:(kh + 1) * 512],
                            start=True, stop=True,
                        )
                    u_sb = workp3.tile([128, S], f32, tag="u")
                    nc.vector.scalar_tensor_tensor(
                        u_sb[:], s_ps[:], SCALE, dist_sb[:, qt, :],
                        op0=mybir.AluOpType.mult, op1=mybir.AluOpType.mult)
                    if use_mask:
                        m_sb = workp.tile([128, S], f32, tag="madd")
                        nc.sync.dma_start(
                            m_sb[:], madd_d[qt * 128:(qt + 1) * 128, :])
                        nc.vector.tensor_add(u_sb[:], u_sb[:], m_sb[:])
                    e_sb = workp3.tile([128, S], f32, tag="e")
                    rowsum = workp3.tile([128, 1], f32, tag="rowsum")
                    nc.scalar.activation(
                        e_sb[:], u_sb[:], mybir.ActivationFunctionType.Exp,
                        accum_out=rowsum[:])
                    # epilogue of the PREVIOUS iteration (skewed by one to
                    # keep DVE from waiting on this iteration's exp)
                    if pend is not None:
                        _h, _qt, _e, _rs, _pb = pend
                        recip = workp3.tile([128, 1], f32, tag="recip")
                        nc.vector.reciprocal(recip[:], _rs[:])
                        p_sb = workp4.tile([128, S], f32, tag="p")
                        nc.vector.tensor_scalar(
                            p_sb[:], _e[:], recip[:], None,
                            op0=mybir.AluOpType.mult)
                        if not (probe & 2):
                            nc.gpsimd.dma_start(
                                out=attn_d[_h, _qt * 128:(_qt + 1) * 128, :],
                                in_=p_sb[:])
                        if not (probe & 1):
                            nc.gpsimd.dma_start(
                                out=_pb[_qt * 128:(_qt + 1) * 128, :],
                                in_=p_sb[:])
                    pend = (h, qt, e_sb, rowsum, pb_dram)
                    # previous-previous head's C^T drains (PV long done)
                    if (probe & 1) and pvq is None:
                        pass
                    if pvq and qt < 2:
                        ct_drain(pvq[0], pvq[1], qt)
                    # interleave previous head's transpose-reads
                    if prev is not None and not (probe & 1):
                        nc.sync.dma_start_transpose(
                            prev[2][:, qt, :],
                            prev[1][:, qt * 128:(qt + 1) * 128])
                if prev is not None and not (probe & 1):
                    cp = pv_for_head(prev[0], prev[2])
                    pvq = (prev[0], cp)
                prev = (h, pb_dram, pt_cur)
            # epilogue: flush last pending iteration
            _h, _qt, _e, _rs, _pb = pend
            recip = workp3.tile([128, 1], f32, tag="recip")
            nc.vector.reciprocal(recip[:], _rs[:])
            p_sb = workp4.tile([128, S], f32, tag="p")
            nc.vector.tensor_scalar(
                p_sb[:], _e[:], recip[:], None, op0=mybir.AluOpType.mult)
            nc.sync.dma_start(
                attn_d[_h, _qt * 128:(_qt + 1) * 128, :], p_sb[:])
            nc.gpsimd.dma_start(
                out=_pb[_qt * 128:(_qt + 1) * 128, :], in_=p_sb[:])
            # last head transposes + PV + remaining drains
            if probe & 1:
                nc.vector.memset(ct_sb[:], 0.0)
            if pvq:
                ct_drain(pvq[0], pvq[1], 0)
                ct_drain(pvq[0], pvq[1], 1)
            if not (probe & 1):
                for kc in range(NK):
                    nc.sync.dma_start_transpose(
                        prev[2][:, kc, :],
                        prev[1][:, kc * 128:(kc + 1) * 128])
                cp = pv_for_head(prev[0], prev[2])
                ct_drain(prev[0], cp, 0)
                ct_drain(prev[0], cp, 1)

            # ---- stage D: output projection + residual ----
            for st in range(NQ):
                op = ps_big.tile([128, S], f32, tag="big")
                for kc in range(NC):
                    nc.tensor.matmul(
                        op[:, 0:512],
                        ct_sb[:, kc, st * 128:(st + 1) * 128],
                        wo_bf[:, kc, :],
                        start=(kc == 0), stop=(kc == NC - 1),
                    )
                xq_res = workp.tile([128, D], f32, tag="xq_res")
                nc.sync.dma_start(
                    xq_res[:], xq_d[st * 128:(st + 1) * 128, :])
                o_sb = workp.tile([128, D], f32, tag="o")
                nc.vector.tensor_add(o_sb[:], op[:, 0:512], xq_res[:])
                nc.sync.dma_start(out_d[st * 128:(st + 1) * 128, :], o_sb[:])

    nc.compile()
    return nc


def _get_nc(use_mask):
    key = ("nc", use_mask)
    if key not in _CACHE:
        _CACHE[key] = build(use_mask)
    return _CACHE[key]


def kernel(input_Q, input_K, input_V, dist_factor, attn_mask,
           W_Q, W_K, W_V, W_O):
    input_Q = np.ascontiguousarray(np.asarray(input_Q, dtype=np.float32))
    input_K = np.ascontiguousarray(np.asarray(input_K, dtype=np.float32))
    input_V = np.ascontiguousarray(np.asarray(input_V, dtype=np.float32))
    dist_factor = np.ascontiguousarray(np.asarray(dist_factor, dtype=np.float32))
    attn_mask = np.asarray(attn_mask)
    W_Q = np.ascontiguousarray(np.asarray(W_Q, dtype=np.float32))
    W_K = np.ascontiguousarray(np.asarray(W_K, dtype=np.float32))
    W_V = np.ascontiguousarray(np.asarray(W_V, dtype=np.float32))
    W_O = np.ascontiguousarray(np.asarray(W_O, dtype=np.float32))

    use_mask = bool(attn_mask.any())
    nc = _get_nc(use_mask)

    in_maps = []
    for c in range(N_CORES):
        m = {
            "xq": input_Q[c], "xk": input_K[c], "xv": input_V[c],
            "dist": dist_factor[c],
            "wq": W_Q, "wk": W_K, "wv": W_V, "wo": W_O,
        }
        if use_mask:
            m["madd"] = np.where(attn_mask[c], np.float32(-1e10),
                                 np.float32(0.0)).astype(np.float32)
        in_maps.append(m)

    res = run_bass_kernel_spmd(nc, in_maps, core_ids=list(range(N_CORES)))
    output = np.stack([res.results[c]["out"] for c in range(N_CORES)])
    attn = np.stack([res.results[c]["attn"] for c in range(N_CORES)])
    return output, attn
